# revision 31
# baseline (speedup 1.0000x reference)
"""Causal self-attention (RMSNorm-QK + RoPE) Trainium2 Bass kernel.

Problem: B=2, T=2048, C=1024, H=16 heads, D=64.
Sharding: 8 cores = 2 (batch) x 4 (head groups of 4 heads).
Each core computes q/k/v projections for its 4 heads, attention, and a
partial output projection (column-parallel over heads); the host sums the
4 partials per batch and transposes.

All matmuls / elementwise run in float16 (tolerance is 2e-2; fp16 keeps the
softmax-exponent error ~1e-3).  PSUM accumulation is fp32.

Layout ("attention layout", channels on partitions, tokens free):
  q/k/v per chunk c in {0,1}: partitions = [head 2c d0..63, head 2c+1 d0..63].
  RoPE pairs (d, d+32) live at partitions p, p^32; the rotate-half operand
  comes from 4 SBUF->SBUF partition-swap DMAs.  BOTH q and k are rms-
  normalized up front (k's normalizer folds in 1/sqrt(D)), so the softmax
  exp has a constant scale and one ACTIVATE covers both heads of a chunk
  through a 3D [128, 2, TB] PSUM view.
  v 16 x [128, 260] f16: head h at cols 65h.., ones col at 65h+64 so the
  PV matmul accumulates the softmax denominator in Y row 64.

Pipeline: emission interleaves attention(j) heads with projection block
j+1 pieces (attention is Scalar-bound, projections Tensor-bound); the
S->exp->PV chain is software-pipelined (S_{k+1} issued before PV_k) so the
PE never waits on the exp; denominators of the 4 heads are batched into one
[4, T-block] Ln/Exp reciprocal on Scalar; out-projection runs one window
behind attention.
"""

import sys

for _p in ("/opt/trn_rl_repo",):
    if _p not in sys.path:
        sys.path.append(_p)

import numpy as np

B, T, C = 2, 2048, 1024
H_TOT, D = 16, 64
HPC = 4               # heads per core
N_CORES = 8
P = 128               # partitions
NB = 4                # t-blocks of 512
TB = 512              # t-block size
KCH = 8               # C / 128 contraction chunks
VW = 65 * HPC         # v width with ones columns = 260
RMS_EPS = 1.1920928955078125e-07
ROPE_BASE = 10000.0

_CACHE = {}


def _patch_act_tables():
    """Restrict ln/exp to the combined act-table set so bass's greedy
    first-match table pass emits a single LoadActFuncSet instead of
    ping-ponging (1.28us per reload).  Set indices are unchanged, only the
    (cached) set contents seen by the placement pass."""
    import concourse.bacc as bacc
    import concourse.hw_specs as hw_specs
    import concourse.mybir as mybir

    if getattr(bacc, "_act_tables_patched", False):
        return
    orig = hw_specs.get_activation_tables

    def patched(arch):
        tabs = dict(orig(arch))
        out = {}
        for name, s in tabs.items():
            s = set(s)
            if name != "natural_log_exp_and_others":
                s.discard(mybir.ActivationFunctionType.Ln)
                s.discard(mybir.ActivationFunctionType.Exp)
            out[name] = s
        return out

    bacc.get_activation_tables = patched
    bacc._act_tables_patched = True


def _build_consts():
    """Host-side constant tensors shared by all cores (fp16)."""
    inv_freq = (1.0 / (ROPE_BASE ** (np.arange(0, D, 2, dtype=np.float32) / np.float32(D)))).astype(np.float32)
    pos = np.arange(T, dtype=np.float32)
    freqs = np.outer(pos, inv_freq).astype(np.float32)      # [T, 32]
    cos = np.cos(freqs).astype(np.float32).T                # [32, T]
    sin = np.sin(freqs).astype(np.float32).T
    # cos2[p] = cos[p%32]; sin2[p] = +sin[p%32] for (p//32)%2==0 else -sin
    cos2 = np.tile(cos, (4, 1)).astype(np.float16)          # [128, T]
    sin2 = np.empty((P, T), dtype=np.float32)
    for g in range(4):
        sgn = 1.0 if g % 2 == 0 else -1.0
        sin2[32 * g:32 * g + 32] = sgn * sin
    sin2 = sin2.astype(np.float16)
    ind2 = np.zeros((P, 2), dtype=np.float16)               # col j: rows 64j..64j+63
    ind2[0:64, 0] = 1.0
    ind2[64:128, 1] = 1.0
    bc64 = np.zeros((2, P), dtype=np.float16)               # row j: cols 64j..
    bc64[0, 0:64] = 1.0
    bc64[1, 64:128] = 1.0
    cossin = np.concatenate([cos2, sin2], axis=1)           # [128, 2T]
    return dict(cossin=cossin, ind2=ind2, bc64=bc64)


def _build_module():
    _patch_act_tables()
    import concourse.bacc as bacc
    import concourse.mybir as mybir
    import concourse.tile as tile

    f32 = mybir.dt.float32
    f16 = mybir.dt.float16
    Exp = mybir.ActivationFunctionType.Exp
    Ln = mybir.ActivationFunctionType.Ln
    Copy = mybir.ActivationFunctionType.Copy
    Alu = mybir.AluOpType

    nc = bacc.Bacc("TRN2", target_bir_lowering=False, debug=False,
                   num_devices=N_CORES)

    xt_d = nc.dram_tensor("xt", [C, T], f16, kind="ExternalInput").ap()
    wqk_d = nc.dram_tensor("wqk", [C, 512], f16, kind="ExternalInput").ap()
    wv_d = nc.dram_tensor("wv", [C, VW], f16, kind="ExternalInput").ap()
    wp_d = nc.dram_tensor("wp", [256, C], f16, kind="ExternalInput").ap()
    cossin_d = nc.dram_tensor("cossin", [P, 2 * T], f16, kind="ExternalInput").ap()
    ind2_d = nc.dram_tensor("ind2", [P, 2], f16, kind="ExternalInput").ap()
    bc64_d = nc.dram_tensor("bc64", [2, P], f16, kind="ExternalInput").ap()
    out_d = nc.dram_tensor("outT", [C, T], f16, kind="ExternalOutput").ap()

    with tile.TileContext(nc) as tc:
        with (
            tc.tile_pool(name="sb", bufs=1) as sb,
            tc.tile_pool(name="tr", bufs=2) as tr,
            tc.tile_pool(name="ps", bufs=2, space="PSUM") as ps,
        ):
            # ---------------- persistent tiles + loads ----------------
            def load(name, dram_slice, shape, dt=f16, eng=None):
                t = sb.tile(shape, dt, tag=name, name=name)
                (eng or nc.sync).dma_start(out=t[:], in_=dram_slice)
                return t

            epsq = sb.tile([2, 1], f32, tag="epsq", name="epsq")
            nc.gpsimd.memset(epsq[:], RMS_EPS)
            epsk = sb.tile([2, 1], f32, tag="epsk", name="epsk")
            nc.gpsimd.memset(epsk[:], 64.0 * RMS_EPS)

            # Startup loads fan out across three DGE queues so the first
            # projection matmul can start ~8us in: SP carries consts + wqk,
            # Scalar carries x blocks 0-1, GpSimd (SWDGE) the rest.
            ind2_t = load("ind2", ind2_d[:, :], [P, 2])
            bc64_t = load("bc64", bc64_d[:, :], [2, P])
            # wq|wk packed in one dram tensor; chunks split across the SP and
            # GpSimd DGE queues so issue time halves and arrival is spread
            wqk_t = [load(f"wqk{k}", wqk_d[k * P:(k + 1) * P, :], [P, 512],
                          eng=(nc.sync if k < 4 else nc.gpsimd))
                     for k in range(KCH)]
            wq_t = [t[:, 0:256] for t in wqk_t]
            wk_t = [t[:, 256:512] for t in wqk_t]
            x_t = [[sb.tile([P, 2 * TB], f16, tag=f"x{k}_{hf}", name=f"x{k}_{hf}")
                    for hf in range(2)] for k in range(KCH)]
            # Startup loads only what window 0 touches (x block-0 halves);
            # the rest of x / wp stream in behind the first projections so
            # the first matmul is not stuck behind 4.5MB of HBM traffic.
            for k in range(KCH):
                nc.scalar.dma_start(out=x_t[k][0][:, 0:TB],
                                    in_=xt_d[k * P:(k + 1) * P, 0:TB])
            wv_t = [load(f"wv{k}", wv_d[k * P:(k + 1) * P, :], [P, VW],
                         eng=nc.gpsimd) for k in range(KCH)]
            cossin_t = load("cossin", cossin_d[:, :], [P, 2 * T],
                            eng=nc.gpsimd)
            wp_t = [sb.tile([P, C], f16, tag=f"wp{c}", name=f"wp{c}")
                    for c in range(2)]

            # persistent intermediates
            rcq_t = [sb.tile([P, T], f16, tag=f"rcq{c}", name=f"rcq{c}")
                     for c in range(2)]
            rck_t = [sb.tile([P, T], f16, tag=f"rck{c}", name=f"rck{c}")
                     for c in range(2)]
            yT_t = [sb.tile([P, T], f16, tag=f"yT{c}", name=f"yT{c}")
                    for c in range(2)]
            v_t = [sb.tile([P, VW], f16, tag=f"v{s}", name=f"v{s}")
                   for s in range(T // P)]

            # ---------------- phase-1 emission closures ----------------
            # Projection matmuls are dependency-free once weights/x land, so
            # they are emitted as small closures drained INTO the attention
            # chunk loop: the PE then always has issueable work between the
            # exp-gated PV matmuls (keeps the HAM clock-gate at 8/8).
            def qk_proj_closures(n, which):
                """16 matmul closures; chunk-final ones add copies/stats.
                Stats (sum of squares -> 1/rms) are computed for BOTH q and k
                (k's fold in 1/sqrt(D)) so the softmax exp scale is constant
                and the two heads' exps batch into one ACTIVATE."""
                hf, tl = n // 2, (n % 2) * TB
                w_t = wq_t if which == "q" else wk_t
                pp = [None, None]
                xm = tr.tile([P, 2 * TB], f16, tag="xm", name=f"x{which}{n}",
                             bufs=2)
                sq = tr.tile([P, 2 * TB], f16, tag="sqm", name=f"sq{which}{n}",
                             bufs=2)
                state = {}

                def stats():
                    msum = ps.tile([2, 2 * TB], f32, tag="ps2",
                                   name=f"ms{which}{n}", bufs=2)
                    for c in range(2):
                        csl = slice(c * TB, (c + 1) * TB)
                        nc.tensor.matmul(msum[:, csl], lhsT=ind2_t[:],
                                         rhs=sq[:, csl], start=True, stop=True)
                    invr = tr.tile([2, 2 * TB], f16, tag="invr",
                                   name=f"ivr{which}{n}", bufs=2)
                    if which == "q":
                        nc.scalar.activation(invr[:], msum[:], Ln,
                                             bias=epsq[:], scale=1.0 / 64.0)
                    else:
                        nc.scalar.activation(invr[:], msum[:], Ln,
                                             bias=epsk[:])
                    nc.scalar.activation(invr[:], invr[:], Exp, scale=-0.5)
                    state[0] = invr

                def mk(c, k):
                    def emit():
                        if k == 0:
                            pp[c] = ps.tile([P, TB], f32, tag="p1",
                                            name=f"p{which}{n}_{c}", bufs=2)
                        nc.tensor.matmul(pp[c][:],
                                         lhsT=w_t[k][:, c * P:(c + 1) * P],
                                         rhs=x_t[k][hf][:, tl:tl + TB],
                                         start=(k == 0), stop=(k == KCH - 1))
                        if k == KCH - 1:
                            nc.vector.tensor_copy(xm[:, c * TB:(c + 1) * TB],
                                                  pp[c][:])
                            csl = slice(c * TB, (c + 1) * TB)
                            nc.vector.tensor_mul(sq[:, csl], xm[:, csl],
                                                 xm[:, csl])
                    return emit

                cls = [mk(c, k) for c in range(2) for k in range(KCH)]
                return cls, stats, xm, sq, state

            def qk_post(n, which, xm, sq, state):
                """rms-normalize + rotate-half swap + rope; emitted at a
                point where the stats chain has long completed."""
                nsl = slice(n * TB, (n + 1) * TB)
                invr = state[0]
                src = tr.tile([P, 2 * TB], f16, tag="xnm",
                              name=f"xn{which}{n}", bufs=2)
                for c in range(2):
                    csl = slice(c * TB, (c + 1) * TB)
                    inv128 = ps.tile([P, TB], f32, tag="p1",
                                     name=f"iv{which}{n}_{c}", bufs=2)
                    nc.tensor.matmul(inv128[:], lhsT=bc64_t[:],
                                     rhs=invr[:, csl], start=True, stop=True)
                    nc.vector.tensor_mul(src[:, csl], xm[:, csl],
                                         inv128[:])
                xsw = tr.tile([P, 2 * TB], f16, tag="xsw",
                              name=f"x{which}sw{n}", bufs=2)
                for g in range(2):
                    a, b = 64 * g, 64 * g + 32
                    nc.sync.dma_start(out=xsw[a:a + 32, :], in_=src[b:b + 32, :])
                    nc.sync.dma_start(out=xsw[b:b + 32, :], in_=src[a:a + 32, :])
                rc_t = rcq_t if which == "q" else rck_t
                for c in range(2):
                    csl = slice(c * TB, (c + 1) * TB)
                    t1 = tr.tile([P, TB], f16, tag="t12",
                                 name=f"t1{which}{n}_{c}", bufs=3)
                    t2 = tr.tile([P, TB], f16, tag="t12",
                                 name=f"t2{which}{n}_{c}", bufs=3)
                    nc.vector.tensor_mul(t1[:], src[:, csl], cossin_t[:, nsl])
                    nc.vector.tensor_mul(t2[:], xsw[:, csl],
                                         cossin_t[:, T + n * TB:T + (n + 1) * TB])
                    nc.vector.tensor_add(rc_t[c][:, nsl], t1[:], t2[:])

            def v_closures(n):
                """8 closures of 4 v matmuls each."""
                hf, tl = n // 2, (n % 2) * TB
                pvs = [None] * 4

                def mk(s_rel, half_k):
                    def emit():
                        if half_k == 0:
                            pvs[s_rel] = ps.tile([P, VW], f32, tag="p1",
                                                 name=f"pv{n}_{s_rel}", bufs=2)
                        for k in range(half_k * 4, half_k * 4 + 4):
                            nc.tensor.matmul(
                                pvs[s_rel][:],
                                lhsT=x_t[k][hf][:, tl + s_rel * P:
                                                tl + (s_rel + 1) * P],
                                rhs=wv_t[k][:], start=(k == 0),
                                stop=(k == KCH - 1))
                        if half_k == 1:
                            vt = v_t[4 * n + s_rel]
                            nc.vector.tensor_copy(vt[:], pvs[s_rel][:])
                            nc.vector.tensor_scalar(vt[:, 64:VW:65],
                                                    pvs[s_rel][:, 64:VW:65],
                                                    0.0, 1.0, Alu.mult, Alu.add)
                    return emit

                return [mk(s, hk) for s in range(4) for hk in range(2)]

            def p4_closures(j, scalar_cast=False):
                """8 closures: out-projection chunk (2 matmuls + copy + DMA).
                For the tail windows the psum->sbuf cast alternates onto the
                otherwise-idle Scalar engine."""
                jsl = slice(j * TB, (j + 1) * TB)

                def mk(o):
                    def emit():
                        osl = slice(o * P, (o + 1) * P)
                        po = ps.tile([P, TB], f32, tag="p1", name=f"po{j}_{o}",
                                     bufs=2)
                        nc.tensor.matmul(po[:], lhsT=wp_t[0][:, osl],
                                         rhs=yT_t[0][:, jsl], start=True,
                                         stop=False)
                        nc.tensor.matmul(po[:], lhsT=wp_t[1][:, osl],
                                         rhs=yT_t[1][:, jsl], start=False,
                                         stop=True)
                        ob = tr.tile([P, TB], f16, tag="ob", name=f"ob{j}_{o}",
                                     bufs=3)
                        if scalar_cast and o % 2 == 0:
                            nc.scalar.activation(ob[:], po[:], Copy)
                        else:
                            nc.vector.tensor_copy(ob[:], po[:])
                        eng = nc.sync if o % 2 == 0 else nc.gpsimd
                        eng.dma_start(out=out_d[osl, jsl], in_=ob[:])
                    return emit

                return [mk(o) for o in range(8)]

            # ---------------- attention ----------------
            def attn_pair(cch, j, den2, queue, final=False):
                """Both heads of chunk cch; S->exp->PV software-pipelined and
                the two heads' stationaries sit in disjoint PE quadrants.
                One queued projection closure drains per chunk."""
                n_k = 4 * (j + 1)
                Ys = [ps.tile([65, TB], f32, tag="py", name=f"Y{cch}_{hh}_{j}",
                              bufs=2) for hh in range(2)]
                pend = [None, None]
                for k in range(n_k):
                    r = k - 4 * j
                    mt = 128 * r if r > 0 else 0
                    S = ps.tile([P, 2, TB], f32, tag="ps2",
                                name=f"S{cch}_{j}_{k}", bufs=2)
                    for hh in range(2):
                        rsl = slice(64 * hh, 64 * hh + 64)
                        nc.tensor.matmul(
                            S[:, hh, mt:],
                            lhsT=rck_t[cch][rsl, k * P:(k + 1) * P],
                            rhs=rcq_t[cch][rsl, j * TB + mt:(j + 1) * TB],
                            start=True, stop=True)
                    # one exp + one mask for BOTH heads via the 3D view
                    e0 = tr.tile([P, 2, TB], f16, tag="e0",
                                 name=f"e{cch}_{j}_{k}", bufs=6)
                    nc.scalar.activation(e0[:, :, mt:], S[:, :, mt:], Exp)
                    if r >= 0:
                        nc.gpsimd.affine_select(
                            out=e0[:, :, 128 * r:128 * r + 128],
                            in_=e0[:, :, 128 * r:128 * r + 128],
                            pattern=[[0, 2], [1, 128]], compare_op=Alu.is_ge,
                            fill=0.0, base=0, channel_multiplier=-1)
                    for hh in range(2):
                        if pend[hh] is not None:
                            pe0, pmt, pk = pend[hh]
                            h = 2 * cch + hh
                            nc.tensor.matmul(
                                Ys[hh][:, pmt:],
                                lhsT=v_t[pk][:, 65 * h:65 * h + 65],
                                rhs=pe0[:, hh, pmt:], start=(pk == 0),
                                stop=False)
                        pend[hh] = (e0, mt, k)
                    # adaptive drain: spread the whole queue evenly over the
                    # remaining chunks so the PE always has filler between
                    # the exp-gated S/PV ops (keeps the HAM clock-gate warm)
                    for _ in range(-(-len(queue) // (n_k - k))):
                        queue.popleft()()
                yraws = []
                for hh in range(2):
                    pe0, pmt, pk = pend[hh]
                    h = 2 * cch + hh
                    nc.tensor.matmul(Ys[hh][:, pmt:],
                                     lhsT=v_t[pk][:, 65 * h:65 * h + 65],
                                     rhs=pe0[:, hh, pmt:], start=(pk == 0),
                                     stop=True)
                    # y rows to sbuf (f32: pre-normalization can be large),
                    # denominator row into the window-shared den4 tile.
                    yraw = tr.tile([65, TB], f32, tag="yrw", name=f"yr{h}_{j}",
                                   bufs=5, padded_shape=[P, TB])
                    nc.vector.tensor_copy(yraw[:], Ys[hh][:])
                    if not final:
                        nc.sync.dma_start(out=den2[hh:hh + 1, :],
                                          in_=yraw[64:65, :])
                    yraws.append(yraw)
                if final:
                    return yraws, Ys
                return yraws

            def attn_tail_final(j, cch, yraws, Ys):
                """Last pair of the kernel: per-head chains with the
                denominator Ln reading Ys row 64 straight from PSUM, skipping
                the den gather DMA (shaves ~2.5us off the closing latency)."""
                jsl = slice(j * TB, (j + 1) * TB)
                for hh in range(2):
                    rsl = slice(64 * hh, 64 * hh + 64)
                    invd = tr.tile([1, TB], f16, tag="invd",
                                   name=f"invf{hh}", bufs=2)
                    nc.scalar.activation(invd[:], Ys[hh][64:65, :], Ln)
                    nc.scalar.activation(invd[:], invd[:], Exp, scale=-1.0)
                    bcD = ps.tile([64, TB], f32, tag="p1",
                                  name=f"bcDf{hh}", bufs=2)
                    nc.tensor.matmul(bcD[:], lhsT=bc64_t[0:1, 0:64],
                                     rhs=invd[:], start=True, stop=True)
                    yn = tr.tile([64, TB], f16, tag="ynm",
                                 name=f"ynf{hh}", bufs=2,
                                 padded_shape=[P, TB])
                    nc.vector.tensor_mul(yn[:], yraws[hh][0:64, :], bcD[:])
                    nc.sync.dma_start(out=yT_t[cch][rsl, jsl], in_=yn[:])

            def attn_tail_pair(j, cch, den2, yraws):
                """Per-pair denominator reciprocal + normalize + yT scatter;
                emitted a few chunks into the NEXT pair/window so the den
                chain has resolved by the time the PE reaches bcD."""
                jsl = slice(j * TB, (j + 1) * TB)
                invd = tr.tile([2, TB], f16, tag="invd", name=f"invd{cch}_{j}",
                               bufs=2)
                nc.scalar.activation(invd[:], den2[:], Ln)
                nc.scalar.activation(invd[:], invd[:], Exp, scale=-1.0)
                for hh in range(2):
                    rsl = slice(64 * hh, 64 * hh + 64)
                    bcD = ps.tile([64, TB], f32, tag="p1",
                                  name=f"bcD{cch}_{hh}_{j}", bufs=2)
                    nc.tensor.matmul(bcD[:],
                                     lhsT=bc64_t[:, 64 * hh:64 * hh + 64],
                                     rhs=invd[:], start=True, stop=True)
                    yn = tr.tile([64, TB], f16, tag="ynm",
                                 name=f"yn{cch}_{hh}_{j}", bufs=2,
                                 padded_shape=[P, TB])
                    nc.vector.tensor_mul(yn[:], yraws[hh][0:64, :], bcD[:])
                    nc.sync.dma_start(out=yT_t[cch][rsl, jsl], in_=yn[:])

            # ---------------- schedule ----------------
            from collections import deque

            # window 0: projections for block 0 (no attention to interleave)
            clsq, stsq, xmq, sqq, stq = qk_proj_closures(0, "q")
            for f in clsq:
                f()
            stsq()
            # x block 1 / blocks 2-3 + wp stream in behind the block-0
            # projections (Scalar/GpSimd DGE) off the startup critical path.
            for k in range(KCH):
                nc.sync.dma_start(out=x_t[k][0][:, TB:2 * TB],
                                  in_=xt_d[k * P:(k + 1) * P, TB:2 * TB])
            clsv0 = v_closures(0)
            # v(0) first halves: PE filler while the q stats/rope chain runs
            for f in clsv0[:4]:
                f()
            qk_post(0, "q", xmq, sqq, stq)
            for k in range(KCH):
                nc.sync.dma_start(out=x_t[k][1][:],
                                  in_=xt_d[k * P:(k + 1) * P, 2 * TB:4 * TB])
            for c in range(2):
                nc.gpsimd.dma_start(out=wp_t[c][:],
                                    in_=wp_d[c * P:(c + 1) * P, :])
            clsk, stsk, xmk, sqk, stk = qk_proj_closures(0, "k")
            for f in clsk:
                f()
            stsk()
            for f in clsv0[4:]:
                f()
            qk_post(0, "k", xmk, sqk, stk)

            # windows 1..NB: attention j = w-1 with projection closures
            # drained into the attention chunk loops; each pair's softmax
            # tail is deferred a few chunks into the following pair/window
            pending_tail = None
            for w in range(1, NB + 1):
                j = w - 1
                den_a = tr.tile([2, TB], f32, tag="den4", name=f"dena_{j}",
                                bufs=2)
                den_b = tr.tile([2, TB], f32, tag="den4", name=f"denb_{j}",
                                bufs=2)
                qA = deque()
                qB = deque()
                pieces = {}
                statsA = statsB = None
                if w < NB:
                    clsq, statsA, xmq, sqq, stq = qk_proj_closures(w, "q")
                    qA.extend(clsq)
                    pieces["q"] = (xmq, sqq, stq)
                # out-projections are deferred toward the tail so the PE has
                # filler during the attention-heavy final windows (HAM warm)
                if w == 3:
                    qA.extend(p4_closures(0))
                if w == NB:
                    qA.extend(v_closures(NB - 1))
                    qB.extend(p4_closures(1))
                    qB.extend(p4_closures(2, scalar_cast=True))
                if w < NB:
                    clsk, statsB, xmk, sqk, stk = qk_proj_closures(w, "k")
                    qB.extend(clsk)
                    pieces["k"] = (xmk, sqk, stk)
                    if w < NB - 1:
                        qB.extend(v_closures(w))
                if statsA is not None:
                    qA.append(statsA)
                if statsB is not None:
                    qB.append(statsB)

                if pending_tail is not None:
                    qA.insert(2, pending_tail)
                yr_a = attn_pair(0, j, den_a, qA)
                qB.insert(min(4, len(qB)),
                          (lambda jj, da, ya: lambda:
                           attn_tail_pair(jj, 0, da, ya))(j, den_a, yr_a))
                for f in qA:
                    f()
                if "q" in pieces:
                    qk_post(w, "q", *pieces["q"])
                if w == NB:
                    yr_b, ys_b = attn_pair(1, j, den_b, qB, final=True)
                    for f in qB:
                        f()
                    attn_tail_final(j, 1, yr_b, ys_b)
                    pending_tail = None
                else:
                    yr_b = attn_pair(1, j, den_b, qB)
                    for f in qB:
                        f()
                    if "k" in pieces:
                        qk_post(w, "k", *pieces["k"])
                    pending_tail = (lambda jj, db, yb: lambda:
                                    attn_tail_pair(jj, 1, db, yb))(j, den_b,
                                                                   yr_b)
            for f in p4_closures(NB - 1, scalar_cast=True):
                f()

    nc.compile()
    return nc


def _get_module():
    if "nc" not in _CACHE:
        _CACHE["nc"] = _build_module()
        _CACHE["consts"] = _build_consts()
    return _CACHE["nc"], _CACHE["consts"]


def _core_inputs(x, w_q, w_k, w_v, w_proj, core):
    """Build the per-core input map (numpy fp16, host-side sharding)."""
    b = core // 4
    g = core % 4
    heads = [4 * g + j for j in range(HPC)]

    xt = np.ascontiguousarray(x[b].T).astype(np.float16)     # [C, T]

    # attention-layout column perm: col m of chunk c -> head 2c+(m//64), dim m%64
    perm = np.empty(256, dtype=np.int64)
    for m in range(256):
        c, mm = m // 128, m % 128
        perm[m] = 64 * heads[2 * c + mm // 64] + (mm % 64)
    wqk = np.concatenate([w_q[perm, :].T, w_k[perm, :].T],
                         axis=1).astype(np.float16)              # [C, 512]

    wv_aug = np.zeros((C, VW), dtype=np.float32)
    for j in range(HPC):
        wv_aug[:, 65 * j:65 * j + 64] = w_v[64 * heads[j]:64 * heads[j] + 64, :].T
    wv = wv_aug.astype(np.float16)

    wp = np.ascontiguousarray(w_proj[:, perm].T).astype(np.float16)  # [256, C]

    return dict(xt=xt, wqk=wqk, wv=wv, wp=wp)


def kernel(x, w_q, w_k, w_v, w_proj, _trace=False, _trace_cores=None):
    from concourse.bass_utils import run_bass_kernel_spmd

    nc, consts = _get_module()
    x = np.asarray(x, dtype=np.float32)
    in_maps = []
    for core in range(N_CORES):
        m = _core_inputs(np.asarray(x), np.asarray(w_q), np.asarray(w_k),
                         np.asarray(w_v), np.asarray(w_proj), core)
        m.update(consts)
        in_maps.append(m)

    res = run_bass_kernel_spmd(nc, in_maps, list(range(N_CORES)),
                               trace=_trace, trace_cores=_trace_cores)
    outs = [res.results[c]["outT"] for c in range(N_CORES)]
    out = np.empty((B, T, C), dtype=np.float32)
    for b in range(B):
        acc = outs[4 * b].astype(np.float32)
        for g in range(1, 4):
            acc = acc + outs[4 * b + g].astype(np.float32)
        out[b] = acc.T
    if _trace:
        kernel._last_exec_time_ns = res.exec_time_ns
        kernel._last_results = res
    return out

